# revision 5
# baseline (speedup 1.0000x reference)
"""Trainium2 Bass kernel for nn_AttnBFAN (batched attention w/ focal re-norm).

Data-parallel over the batch dim: 128 batches sharded 16-per-core across 8
NeuronCores. Per batch (Q=128, C=1024, D=1024):
    attn = leaky_relu(context @ query^T, 0.1)          (C, Q)
    attn = attn / (||attn||_2 over q)                  l2norm per (b, c)
    p    = softmax(20 * attn^T, axis=c)                (Q, C)
    t    = (p > mean_c p) * p ; re_attn = t / sum_c t
    wcontext = re_attn @ context                       (Q, D)
returns (query, wcontext, re_attn).

v3: bf16 matmul path. Host pre-casts context/query to bf16 (halves HBM
traffic; rel err vs fp32 reference ~5e-3, dominated by the bmm1 bf16
rounding) and pre-transposes query to [d, q] chunks so no on-chip qT
transposes are needed. All PE transposes and bmms run at 1.0 cycle/row
(bf16) instead of 1.5 (f32r transpose) / 4.0 (f32). The l2-norm /
softmax / focal chain stays f32: per-c-column norm errors do not cancel
in the softmax, so sq/S/ln/exp are kept at f32(r) precision. bmm2
multiplies the *unnormalized* focal weights t (bf16) and folds the
1/sum_c(t) renormalization into the PSUM eviction as a per-partition
ACT scale. Next batch's context transposes are software-pipelined into
this batch's softmax gap to keep the PE busy.
"""

import os
import numpy as np
import ml_dtypes

import concourse.bacc as bacc
import concourse.mybir as mybir
import concourse.tile as tile
from concourse.bass_utils import run_bass_kernel_spmd
from concourse.masks import make_identity
from concourse.hw_specs import get_activation_tables

F32 = mybir.dt.float32
F32R = mybir.dt.float32r
BF16 = mybir.dt.bfloat16
AX = mybir.AxisListType
ALU = mybir.AluOpType
ACTF = mybir.ActivationFunctionType

NCORES = 8
NB = 128          # total batches
BPC = NB // NCORES  # batches per core
Q = 128
C = 1024
D = 1024
SMOOTH = 20.0

_CACHE = {}


def _build():
    nc = bacc.Bacc("TRN2", target_bir_lowering=False, debug=False,
                   num_devices=NCORES, name="attn_bfan")
    # query pre-transposed+tiled on host: [b, p(=d%128), jd, q] bf16
    q_in = nc.dram_tensor("query", [BPC, 128, 8, Q], BF16, kind="ExternalInput")
    c_in = nc.dram_tensor("context", [BPC, C, D], BF16, kind="ExternalInput")
    re_out = nc.dram_tensor("re_attn", [BPC, Q, C], F32, kind="ExternalOutput")
    wc_out = nc.dram_tensor("wcontext", [BPC, Q, D], F32, kind="ExternalOutput")

    with tile.TileContext(nc) as tc:
        with (
            tc.tile_pool(name="singles", bufs=1) as singles,
            tc.tile_pool(name="ctxp", bufs=3) as ctxp,
            tc.tile_pool(name="ctxtp", bufs=2) as ctxtp,
            tc.tile_pool(name="qp", bufs=2) as qp,
            tc.tile_pool(name="work", bufs=2) as work,
            tc.tile_pool(name="w1", bufs=1) as w1,
            tc.tile_pool(name="tp", bufs=2) as tpool,
            tc.tile_pool(name="stat", bufs=2) as stat,
            tc.tile_pool(name="ps_a", bufs=1, space="PSUM") as ps_a,
            tc.tile_pool(name="ps_w", bufs=1, space="PSUM") as ps_w,
            tc.tile_pool(name="ps_f", bufs=1, space="PSUM") as ps_f,
            tc.tile_pool(name="ps_tp", bufs=2, space="PSUM") as ps_tp,
        ):
            tab_names = list(get_activation_tables("gen3").keys())
            nc.scalar.add_instruction(mybir.InstLoadActFuncSet(
                name=nc.get_next_instruction_name(),
                act_func_set_id=tab_names.index("natural_log_exp_and_others"),
                ins=[], outs=[]))
            ident = singles.tile([128, 128], F32, tag="ident")
            make_identity(nc, ident[:])
            identb = singles.tile([128, 128], BF16, tag="identb")
            nc.vector.tensor_copy(identb[:], ident[:])
            ones_f = singles.tile([128, 128], F32, tag="ones_f")
            nc.vector.memset(ones_f[:], 1.0)
            ones_r = singles.tile([128, 128], F32R, tag="ones_r")
            nc.vector.tensor_copy(ones_r[:], ones_f[:])
            ln20 = singles.tile([128, 1], F32, tag="ln20")
            nc.vector.memset(ln20[:], float(np.log(SMOOTH)))

            # software pipeline state: context tiles + their transposes for
            # the *next* batch are produced while the current batch runs its
            # softmax chain.
            ctx_t = [None] * (BPC + 1)   # plain ctx bf16 [128, 8jc, 1024d]
            ctxT_t = [None] * (BPC + 1)  # ctx^T bf16 [128, 8jd, 1024c]

            def load_ctx(b):
                ctx = ctxp.tile([128, 8, D], BF16, tag="ctx", name="ctx")
                ctx_src = c_in[b].rearrange("(jc p) d -> p jc d", p=128)
                nc.gpsimd.dma_start(out=ctx[:, 0:4, :], in_=ctx_src[:, 0:4, :])
                nc.gpsimd.dma_start(out=ctx[:, 4:8, :], in_=ctx_src[:, 4:8, :])
                ctx_t[b] = ctx

            def transpose_ctx(b, jds):
                # PE-transpose ctx [c,d] -> ctxT [d,c] for the given d-chunks.
                ctx = ctx_t[b]
                if ctxT_t[b] is None:
                    ctxT_t[b] = ctxtp.tile([128, 8, C], BF16, tag="ctxT",
                                           name="ctxT")
                ctxT = ctxT_t[b]
                for jd in jds:
                    tp = ps_tp.tile([128, 8, 128], BF16, tag="tp")
                    for jc in range(8):
                        nc.tensor.transpose(
                            tp[:, jc, :],
                            ctx[:, jc, jd * 128:(jd + 1) * 128], identb[:])
                    # one PSUM->SBUF copy per d-chunk (1024 bf16 elems)
                    src = tp[:].rearrange("p a b -> p (a b)")
                    if jd % 4 != 3:
                        nc.vector.tensor_copy(ctxT[:, jd, :], src)
                    else:
                        nc.scalar.copy(ctxT[:, jd, :], src)

            # prologue: batch 0 ctx + transposes
            load_ctx(0)
            transpose_ctx(0, range(8))

            for b in range(BPC):
                qT = qp.tile([128, 8, Q], BF16, tag="qT")
                nc.gpsimd.dma_start(out=qT[:], in_=q_in[b])
                if b + 1 < BPC:
                    load_ctx(b + 1)
                ctx = ctx_t[b]
                ctxT = ctxT_t[b]

                # ---- bmm1: attn^T (q, c) accumulated over 8 d-chunks ----
                a0 = ps_a.tile([128, 512], F32, tag="a0")
                a1 = ps_a.tile([128, 512], F32, tag="a1")
                for jd in range(8):
                    st, sp = jd == 0, jd == 7
                    nc.tensor.matmul(a0[:], qT[:, jd, :], ctxT[:, jd, 0:512],
                                     start=st, stop=sp)
                    nc.tensor.matmul(a1[:], qT[:, jd, :], ctxT[:, jd, 512:1024],
                                     start=st, stop=sp)

                # overlap: first half of next batch's ctx transposes
                if b + 1 < BPC:
                    transpose_ctx(b + 1, range(0, 4))

                # ---- leaky relu via Prelu(alpha=0.1), PSUM -> SBUF ----
                attn = work.tile([128, C], F32, tag="attn")
                nc.scalar.activation(attn[:, 0:512], a0[:], ACTF.Prelu,
                                     bias=0.0, scale=1.0, alpha=0.1)
                nc.scalar.activation(attn[:, 512:1024], a1[:], ACTF.Prelu,
                                     bias=0.0, scale=1.0, alpha=0.1)

                # ---- l2 norm over q: square, ones-matmul (sums over q AND
                #      broadcasts to 128 partitions), 20/sqrt via ln/exp ----
                sq = w1.tile([128, C], F32R, tag="w1a")
                nc.vector.tensor_mul(sq[:], attn[:], attn[:])
                s0 = ps_a.tile([128, 512], F32, tag="a0")
                s1 = ps_a.tile([128, 512], F32, tag="a1")
                nc.tensor.matmul(s0[:], ones_r[:], sq[:, 0:512], start=True, stop=True)
                nc.tensor.matmul(s1[:], ones_r[:], sq[:, 512:1024], start=True, stop=True)

                # overlap: second half of next batch's ctx transposes
                if b + 1 < BPC:
                    transpose_ctx(b + 1, range(4, 8))

                # 20/sqrt(S) = exp(-0.5*ln(S) + ln 20)
                lnS = w1.tile([128, C], F32, tag="w1b")
                nc.scalar.activation(lnS[:, 0:512], s0[:], ACTF.Ln)
                nc.scalar.activation(lnS[:, 512:1024], s1[:], ACTF.Ln)
                rn20 = w1.tile([128, C], F32, tag="w1c")
                nc.scalar.activation(rn20[:], lnS[:], ACTF.Exp,
                                     bias=ln20[:], scale=-0.5)
                u = w1.tile([128, C], F32, tag="w1a")
                nc.vector.tensor_mul(u[:], attn[:], rn20[:])

                # ---- softmax (no max-sub; |u*20| <= 20) with fused row-sum ----
                pu = work.tile([128, C], F32, tag="pu")
                rs = stat.tile([128, 1], F32, tag="rs")
                nc.scalar.activation(pu[:], u[:], ACTF.Exp,
                                     bias=0.0, scale=1.0, accum_out=rs[:])

                # ---- focal threshold: t = (pu > rs/C) * pu, ts = sum_c t ----
                thr = stat.tile([128, 1], F32, tag="thr")
                nc.scalar.mul(thr[:], rs[:], 1.0 / C)
                t = tpool.tile([128, C], BF16, tag="t")
                ts = stat.tile([128, 1], F32, tag="ts")
                nc.vector.scalar_tensor_tensor(
                    out=t[:], in0=pu[:], scalar=thr[:], in1=pu[:],
                    op0=ALU.is_gt, op1=ALU.mult, accum_out=ts[:])
                rinv = stat.tile([128, 1], F32, tag="rinv")
                nc.vector.reciprocal(rinv[:], ts[:])
                # re_attn = t * (1/ts): ACT copy with per-partition scale
                re = work.tile([128, C], F32, tag="re")
                nc.scalar.activation(re[:], t[:], ACTF.Copy, bias=0.0, scale=rinv[:])
                nc.sync.dma_start(out=re_out[b], in_=re[:])

                # ---- t^T for bmm2 (bf16 transposes) ----
                tT = qp.tile([128, 8, Q], BF16, tag="tT")
                tpf = ps_f.tile([128, 8, 128], BF16, tag="tpf")
                for jc in range(8):
                    nc.tensor.transpose(
                        tpf[:, jc, :],
                        t[:, jc * 128:(jc + 1) * 128], identb[:])
                nc.vector.tensor_copy(
                    tT[:].rearrange("p a b -> p (a b)"),
                    tpf[:].rearrange("p a b -> p (a b)"))

                # ---- bmm2: wcontext (q, d) = sum_c t[q,c] ctx[c,d], then
                #      scaled by rinv on eviction ----
                w0 = ps_w.tile([128, 512], F32, tag="w0")
                w2 = ps_w.tile([128, 512], F32, tag="w1")
                for jc in range(8):
                    st, sp = jc == 0, jc == 7
                    nc.tensor.matmul(w0[:], tT[:, jc, :], ctx[:, jc, 0:512],
                                     start=st, stop=sp)
                    nc.tensor.matmul(w2[:], tT[:, jc, :], ctx[:, jc, 512:1024],
                                     start=st, stop=sp)
                wc = work.tile([128, D], F32, tag="wc")
                nc.scalar.activation(wc[:, 0:512], w0[:], ACTF.Copy,
                                     bias=0.0, scale=rinv[:])
                nc.scalar.activation(wc[:, 512:1024], w2[:], ACTF.Copy,
                                     bias=0.0, scale=rinv[:])
                nc.sync.dma_start(out=wc_out[b], in_=wc[:])
                ctx_t[b] = None
                ctxT_t[b] = None

    nc.compile()
    return nc


def kernel(query: np.ndarray, context: np.ndarray):
    query = np.ascontiguousarray(query, dtype=np.float32)
    context = np.ascontiguousarray(context, dtype=np.float32)
    assert query.shape == (NB, Q, D) and context.shape == (NB, C, D)

    if "nc" not in _CACHE:
        _CACHE["nc"] = _build()
    nc = _CACHE["nc"]

    bf16 = ml_dtypes.bfloat16
    # qT host prep: (B, Q, D) -> [b, p, jd, q] where d = jd*128 + p
    qT = np.ascontiguousarray(
        query.transpose(0, 2, 1).reshape(NB, 8, 128, Q).transpose(0, 2, 1, 3)
    ).astype(bf16)
    ctx_bf = context.astype(bf16)

    in_maps = []
    for k in range(NCORES):
        sl = slice(k * BPC, (k + 1) * BPC)
        in_maps.append({"query": qT[sl], "context": ctx_bf[sl]})

    trace = os.environ.get("KERNEL_TRACE", "0") == "1"
    res = run_bass_kernel_spmd(nc, in_maps, core_ids=list(range(NCORES)),
                               trace=trace)
    _CACHE["last_res"] = res

    re_attn = np.concatenate([r["re_attn"] for r in res.results], axis=0)
    wcontext = np.concatenate([r["wcontext"] for r in res.results], axis=0)
    return query, wcontext, re_attn
